# revision 15
# baseline (speedup 1.0000x reference)
"""Grouped-Query Attention (16 q heads, 4 kv heads, head_dim 128, seq 4096,
hidden 2048) on 8 Trainium2 NeuronCores.

Sharding: sequence-parallel over query tokens (512 per core). Each core
projects q/k/v for its own 512 tokens in bf16; the per-core K^T/V blocks are
AllGathered per kv-head group (4 small collectives, pipelined against
attention on earlier groups), then each core runs full attention for its 512
query rows over all 4096 keys and applies the full output projection,
producing its 512-row slice of the output directly (no reduce needed).

All matmul operands are bf16 (1 cycle/row on the PE array vs 4 for fp32);
accumulation stays fp32 in PSUM. Softmax runs without max-subtraction
(scores are bounded ~|3|): scores are built transposed (S^T[k, q]) per head
pair into one 2-bank PSUM tile and exp'd in a single scalar-engine activation
per pair. The denominator Z = sum_k exp accumulates on the vector engine
(freeing the PE) with a final 128-way partition reduction as one small
matmul; the normalization is a matmul-broadcast of 1/Z folded into the
PSUM->SBUF copy.
"""

import numpy as np
import ml_dtypes

import concourse.bass as bass
import concourse.bacc as bacc
import concourse.tile as tile
from concourse import mybir
from concourse.bass_utils import run_bass_kernel_spmd

# Problem constants
S = 4096          # sequence length
HID = 2048        # hidden dim
NH = 16           # query heads
NKV = 4           # kv heads
D = 128           # head dim
G = NH // NKV     # q heads per kv head (4)
NC = 8            # cores
SC = S // NC      # tokens per core (512)
P = 128           # partitions
KT = HID // P     # contraction tiles over hidden (16)
INV_NORM = 1.0 / float(np.sqrt(D))

FP = mybir.dt.float32
BF = mybir.dt.bfloat16
BF_NP = ml_dtypes.bfloat16


def build_bass():
    nc = bacc.Bacc(None, num_devices=NC)

    # ---- I/O (all activations/weights pre-cast to bf16 on host) ----
    xTc = nc.declare_dram_parameter("xTc", [HID, SC], BF, isOutput=False)
    # wq pre-tiled on host: [P, NH*KT*D] where col ((o*KT)+h)*D+d holds
    # WqT[h*P+p, o*D+d]
    wq = nc.declare_dram_parameter("wq", [P, NH * KT * D], BF, isOutput=False)
    wk = nc.declare_dram_parameter("wk", [HID, NKV * D], BF, isOutput=False)
    wv = nc.declare_dram_parameter("wv", [HID, NKV * D], BF, isOutput=False)
    # wo pre-tiled on host: [2 halves, 16 k-tiles, 128 o, 1024 m]
    wo = nc.declare_dram_parameter("wo", [2, KT, P, HID // 2], BF, isOutput=False)
    y = nc.declare_dram_parameter("y", [SC, HID], FP, isOutput=True)

    # ---- per-kv-group K^T/V collectives (pipelined against attention) ----
    # kv_loc[g] flat [2, D*SC]: [0] = K^T_g (d-major: d*SC + t),
    #                           [1] = V_g  (t-major: t*D + d)
    kv_loc = [nc.dram_tensor(f"kv_loc{g}", [2, D * SC], BF) for g in range(NKV)]
    kv_gath = [nc.dram_tensor(f"kv_gath{g}", [NC, 2, D * SC], BF,
                              addr_space="Shared") for g in range(NKV)]
    groups = [list(range(NC))]

    with tile.TileContext(nc) as tc:
        with (
            tc.tile_pool(name="const", bufs=1) as const_pool,
            tc.tile_pool(name="qt", bufs=1) as qt_pool,
            tc.tile_pool(name="attn_out", bufs=1) as att_pool,
        ):
            ones_k = const_pool.tile([P, 1], FP)      # Z partition-reduce lhsT
            nc.vector.memset(ones_k[:], 1.0)
            ones_m = const_pool.tile([1, P], BF)      # broadcast lhsT (K=1)
            nc.vector.memset(ones_m[:], 1.0)

            qT_sb = qt_pool.tile([P, NH, SC], BF)           # 2 MB
            attT_sb = att_pool.tile([P, NH, SC], BF)        # 2 MB

            # ---------- Phase 1: local projections ----------
            with (
                tc.tile_pool(name="xw", bufs=1) as xw_pool,
                tc.tile_pool(name="proj_psum", bufs=3, space="PSUM") as pj_psum,
                tc.tile_pool(name="proj_sb", bufs=3) as pj_sb,
                tc.tile_pool(name="wq_sb", bufs=2) as wq_pool,
            ):
                xTc_sb = xw_pool.tile([P, KT, SC], BF)          # 2 MB
                wk_sb = xw_pool.tile([P, KT, NKV * D], BF)      # 1 MB
                wv_sb = xw_pool.tile([P, KT, NKV * D], BF)      # 1 MB
                nc.sync.dma_start(
                    out=xTc_sb[:], in_=xTc[:].rearrange("(h p) c -> p h c", p=P))
                nc.sync.dma_start(
                    out=wk_sb[:], in_=wk[:].rearrange("(h p) c -> p h c", p=P))
                nc.sync.dma_start(
                    out=wv_sb[:], in_=wv[:].rearrange("(h p) c -> p h c", p=P))

                # v local first (all groups share the 4 token-tile matmuls)
                vsb = []
                for st in range(SC // P):
                    ps = pj_psum.tile([P, NKV * D], FP)
                    for h in range(KT):
                        nc.tensor.matmul(
                            ps[:],
                            xTc_sb[:, h, st * P:(st + 1) * P],
                            wv_sb[:, h, :],
                            start=(h == 0), stop=(h == KT - 1),
                        )
                    sb = pj_sb.tile([P, NKV * D], BF, name="vsb", tag="vsb",
                                    bufs=4)
                    nc.vector.tensor_copy(sb[:], ps[:])
                    vsb.append(sb)

                # k^T per group, then immediately gather that group's K+V
                for g in range(NKV):
                    ps = pj_psum.tile([P, SC], FP)
                    for h in range(KT):
                        nc.tensor.matmul(
                            ps[:],
                            wk_sb[:, h, g * D:(g + 1) * D],
                            xTc_sb[:, h, :],
                            start=(h == 0), stop=(h == KT - 1),
                        )
                    sb = pj_sb.tile([P, SC], BF)
                    nc.vector.tensor_copy(sb[:], ps[:])
                    nc.sync.dma_start(
                        out=kv_loc[g][0].rearrange("(p c) -> p c", p=P),
                        in_=sb[:])
                    for st in range(SC // P):
                        nc.sync.dma_start(
                            out=kv_loc[g][1]
                            .rearrange("(s p d) -> p s d", p=P, s=SC // P)
                            [:, st, :],
                            in_=vsb[st][:, g * D:(g + 1) * D])
                    nc.gpsimd.collective_compute(
                        "AllGather", mybir.AluOpType.bypass,
                        replica_groups=groups,
                        ins=[kv_loc[g][:]], outs=[kv_gath[g][:]],
                    )

                # q^T: [NH*D, SC] ; lhsT = wq tile [P, D], rhs = xTc tile
                CH = 4  # heads per wq chunk
                for c in range(NH // CH):
                    wqc = wq_pool.tile([P, CH, KT, D], BF)
                    nc.sync.dma_start(
                        out=wqc[:],
                        in_=wq[:, c * CH * KT * D:(c + 1) * CH * KT * D]
                        .rearrange("p (o h d) -> p o h d", o=CH, h=KT),
                    )
                    for ol in range(CH):
                        o = c * CH + ol
                        ps = pj_psum.tile([P, SC], FP)
                        for h in range(KT):
                            nc.tensor.matmul(
                                ps[:], wqc[:, ol, h, :], xTc_sb[:, h, :],
                                start=(h == 0), stop=(h == KT - 1),
                            )
                        nc.vector.tensor_copy(qT_sb[:, o, :], ps[:])

            # ---------- Phase 2+3: attention per kv group ----------
            SK = S // P   # 32 key tiles
            W2 = 2 * SC   # pair width (1024)
            with (
                tc.tile_pool(name="kv_sb", bufs=2) as kv_pool,
                tc.tile_pool(name="st_psum", bufs=2, space="PSUM") as st_psum,
                tc.tile_pool(name="av_psum", bufs=2, space="PSUM") as av_psum,
                tc.tile_pool(name="p_sb", bufs=3) as p_pool,
                tc.tile_pool(name="z_sb", bufs=2) as zs_pool,
            ):
                for g in range(NKV):
                    kT_g = kv_pool.tile([P, NC, SC], BF, tag="kt")   # 1 MB
                    nc.sync.dma_start(
                        out=kT_g[:],
                        in_=kv_gath[g][:, 0, :]
                        .rearrange("j (p c) -> p j c", p=P),
                    )
                    v_g = kv_pool.tile([P, NC, 4, D], BF, tag="v")   # 1 MB
                    for j in range(NC):
                        nc.sync.dma_start(
                            out=v_g[:, j],
                            in_=kv_gath[g][j, 1, :]
                            .rearrange("(s p d) -> p s d", p=P),
                        )

                    for hp in range(G // 2):  # head pairs within group
                        h0 = g * G + 2 * hp
                        av = av_psum.tile([P, W2], FP, name="av", tag="av")
                        z_acc = zs_pool.tile([P, W2], FP, name="z_acc",
                                             tag="z_acc")
                        # software-pipelined: score(sk) issued one step ahead
                        # of av(sk-1) so the PE never waits on the fresh exp
                        pt_q = []

                        def consume(psk, pp):
                            if psk == 0:
                                nc.vector.tensor_copy(z_acc[:], pp[:])
                            else:
                                nc.vector.tensor_add(z_acc[:], z_acc[:], pp[:])
                            for hl in range(2):
                                nc.tensor.matmul(
                                    av[:, hl * SC:(hl + 1) * SC],
                                    v_g[:, psk // 4, psk % 4, :],
                                    pp[:, hl * SC:(hl + 1) * SC],
                                    start=(psk == 0), stop=(psk == SK - 1),
                                )

                        for sk in range(SK):
                            stp = st_psum.tile([P, W2], FP)
                            kblk = kT_g[:, sk // 4, (sk % 4) * P:(sk % 4 + 1) * P]
                            nc.tensor.matmul(
                                stp[:, 0:SC], kblk, qT_sb[:, h0, :],
                                start=True, stop=True,
                            )
                            nc.tensor.matmul(
                                stp[:, SC:W2], kblk, qT_sb[:, h0 + 1, :],
                                start=True, stop=True,
                            )
                            ptile = p_pool.tile([P, W2], BF)
                            nc.scalar.activation(
                                ptile[:], stp[:],
                                mybir.ActivationFunctionType.Exp,
                                scale=INV_NORM,
                            )
                            pt_q.append((sk, ptile))
                            if sk > 0:
                                consume(*pt_q.pop(0))
                        consume(*pt_q.pop(0))

                        # Z partition-reduce + normalize per head
                        for hl in range(2):
                            zp = st_psum.tile([1, SC], FP, name="zp", tag="stp")
                            nc.tensor.matmul(
                                zp[:], ones_k[:],
                                z_acc[:, hl * SC:(hl + 1) * SC],
                                start=True, stop=True,
                            )
                            zr = zs_pool.tile([1, SC], FP, name="zr", tag="zr")
                            nc.vector.reciprocal(zr[:], zp[:])
                            zr16 = zs_pool.tile([1, SC], BF, name="zr16",
                                                tag="zr16")
                            nc.vector.tensor_copy(zr16[:], zr[:])
                            bc = st_psum.tile([P, SC], FP, name="bc", tag="stp")
                            nc.tensor.matmul(
                                bc[:], ones_m[:], zr16[:],
                                start=True, stop=True,
                            )
                            bcs = zs_pool.tile([P, SC], FP, name="bcs", tag="bcs")
                            nc.vector.tensor_copy(bcs[:], bc[:])
                            nc.vector.tensor_mul(
                                attT_sb[:, h0 + hl, :],
                                av[:, hl * SC:(hl + 1) * SC], bcs[:],
                            )

            # ---------- Phase 4: output projection ----------
            MT = SC // P  # 4 query-row tiles
            with (
                tc.tile_pool(name="wo_sb", bufs=3) as wo_pool,
                tc.tile_pool(name="y_psum", bufs=8, space="PSUM") as y_psum,
                tc.tile_pool(name="y_sb", bufs=3) as ys_pool,
            ):
                NW = HID // 2 // 512  # 2 moving chunks of 512 per half
                for half in range(2):
                    ps = [[y_psum.tile([P, 512], FP, name="yp", tag="yp")
                           for _ in range(NW)] for _ in range(MT)]
                    for k in range(KT):
                        wot = wo_pool.tile([P, HID // 2], BF)
                        nc.sync.dma_start(out=wot[:], in_=wo[half, k])
                        for m in range(MT):
                            for n in range(NW):
                                nc.tensor.matmul(
                                    ps[m][n][:],
                                    attT_sb[:, k, m * P:(m + 1) * P],
                                    wot[:, n * 512:(n + 1) * 512],
                                    start=(k == 0), stop=(k == KT - 1),
                                )
                    for m in range(MT):
                        ysb = ys_pool.tile([P, HID // 2], FP)
                        for n in range(NW):
                            nc.vector.tensor_copy(
                                ysb[:, n * 512:(n + 1) * 512], ps[m][n][:],
                            )
                        nc.sync.dma_start(
                            out=y[m * P:(m + 1) * P,
                                  half * (HID // 2):(half + 1) * (HID // 2)],
                            in_=ysb[:],
                        )
    # bacc lowering: splits multi-sem waits (HW allows 1 wait/instruction),
    # moves matmul waits onto LDWEIGHTS, register alloc.
    nc.compile()
    return nc


_CACHED = {}


def _prep_inputs(x, Wq, Wk, Wv, Wo):
    xs = np.ascontiguousarray(x.reshape(S, HID)).astype(np.float32)
    xT = np.ascontiguousarray(xs.T).astype(BF_NP)        # [HID, S]
    wqT = np.ascontiguousarray(Wq.T).astype(BF_NP)       # [HID, NH*D]
    # wq tiled: [P, NH*KT*D] with col ((o*KT)+h)*D+d = wqT[h*P+p, o*D+d]
    wq_t = np.empty((P, NH * KT * D), BF_NP)
    for o in range(NH):
        for h in range(KT):
            c0 = (o * KT + h) * D
            wq_t[:, c0:c0 + D] = wqT[h * P:(h + 1) * P, o * D:(o + 1) * D]
    wkT = np.ascontiguousarray(Wk.T).astype(BF_NP)       # [HID, NKV*D]
    wvT = np.ascontiguousarray(Wv.T).astype(BF_NP)
    woT = np.ascontiguousarray(Wo.T).astype(BF_NP)       # [HID(o), HID(m)]
    wo_t = np.empty((2, KT, P, HID // 2), BF_NP)
    for half in range(2):
        for k in range(KT):
            wo_t[half, k] = woT[k * P:(k + 1) * P,
                                half * (HID // 2):(half + 1) * (HID // 2)]
    in_maps = []
    for c in range(NC):
        in_maps.append({
            "xTc": np.ascontiguousarray(xT[:, c * SC:(c + 1) * SC]),
            "wq": wq_t, "wk": wkT, "wv": wvT, "wo": wo_t,
        })
    return in_maps


def run(x, Wq, Wk, Wv, Wo, trace=False):
    if "nc" not in _CACHED:
        _CACHED["nc"] = build_bass()
    nc = _CACHED["nc"]
    in_maps = _prep_inputs(x, Wq, Wk, Wv, Wo)
    res = run_bass_kernel_spmd(nc, in_maps, list(range(NC)), trace=trace)
    out = np.concatenate([res.results[c]["y"] for c in range(NC)], axis=0)
    return out.reshape(1, S, HID), res


def kernel(x, Wq, Wk, Wv, Wo):
    out, _ = run(np.asarray(x), np.asarray(Wq), np.asarray(Wk),
                 np.asarray(Wv), np.asarray(Wo))
    return out
